# revision 7
# baseline (speedup 1.0000x reference)
"""Trainium2 Bass kernel for nn_Attention_C (XCA-style channel attention).

Pipeline per image: 1x1 conv (GEMM) -> depthwise 3x3 conv -> per-head
l2norm + channel-attention (48x48 Gram over 4096 pixels) -> softmax ->
attn @ v -> 1x1 out-projection.

Sharding: data-parallel over batch. 16 images / 8 cores = 2 images/core.

Engine split (per core):
  - q,k path: GEMM drains write fp8 padded slots; depthwise conv runs on
    TensorE as fp8 DoubleRow diagonal matmuls (2 taps per pass), drained
    by strided-PSUM activations straight into compact bf16 tiles.
  - v path: conv on DVE (tensor_tensor adds, 2x bf16 mode) with the
    per-tap multiplies produced by ScalarE (activation scale) and GpSimd
    (tensor_scalar), writing natural channel-layout vc tiles.
  - attention tail: pair-packed Gram ([96,96] PSUM per head pair),
    softmax with a constant -|temp| shift instead of a max-reduce,
    block-sparse A^T matmuls in the natural channel layout, y written
    bf16 (host casts to f32 and adds b_out).
"""

import os
import sys
import types

import numpy as np

_REPO = "/opt/trn_rl_repo"
if _REPO not in sys.path:
    sys.path.insert(0, _REPO)

# ---------------------------------------------------------------------------
# antenv.axon_hooks shim (the image's antenv lacks it; needed for trace=True)
# ---------------------------------------------------------------------------
if "antenv.axon_hooks" not in sys.modules:
    try:
        from trn_agent_boot.trn_boot import _ntff_profile_via_ctypes

        _hook = _ntff_profile_via_ctypes("/opt/axon/libaxon_pjrt.so")
    except Exception:
        _hook = None
    _m = types.ModuleType("antenv.axon_hooks")
    _m.get_axon_ntff_profile_hook = lambda: _hook
    _m.set_axon_ntff_profile_hook = lambda h: None
    sys.modules["antenv.axon_hooks"] = _m

import ml_dtypes  # noqa: E402
import bass_rust  # noqa: E402
import concourse.bass as bass  # noqa: E402
import concourse.mybir as mybir  # noqa: E402
import concourse.tile as tile  # noqa: E402
from concourse.bass_utils import run_bass_kernel_spmd  # noqa: E402
from concourse.masks import make_identity  # noqa: E402

BF16 = mybir.dt.bfloat16
F32 = mybir.dt.float32
FP8 = mybir.dt.float8e4
AF = mybir.ActivationFunctionType
ALU = mybir.AluOpType
AX = mybir.AxisListType
PM = mybir.MatmulPerfMode

# ---------------------------------------------------------------------------
# Patch TileContext._drain_and_barrier: this walrus build rejects >1 sync
# waits on a CTRL-class (Drain) instruction; split them into standalone waits.
# ---------------------------------------------------------------------------
_MAX_DRAIN_WAITS = 1


def _split_drain_and_barrier(self, tick_clock, wait_clock):
    from concourse.tile import ScopedClock

    nc = self.nc
    drain_inst = nc.sync.drain()
    wait_clock.add_sem_waits(
        drain_inst.ins, ScopedClock({None: tick_clock.global_clock})
    )
    waits = list(drain_inst.ins.sync_info.on_wait)
    if len(waits) > _MAX_DRAIN_WAITS:
        assert self.sems is not None
        by_num = {h.num: h for h in self.sems.allocated().values()}
        keep, spill = [], []
        for w in waits:
            if w.sync_type == "semaphore" and w.id in by_num:
                spill.append(w)
            else:
                keep.append(w)
        while spill and len(keep) < _MAX_DRAIN_WAITS:
            keep.append(spill.pop())
        drain_inst.ins.sync_info = bass_rust.SyncInfo(on_wait=keep, on_update=[])
        for w in spill:
            nc.sync.wait_ge(by_num[w.id], int(w.wait_value))

    nc.all_engine_barrier()
    assert self.sems is not None
    popped = nc._tile_sem_poison_stack.pop()
    assert popped is self._sem_poison
    nc.clear_and_free_semaphores(list(self.sems.allocated().values()))
    nc.all_engine_barrier()


tile.TileContext._drain_and_barrier = _split_drain_and_barrier


def _split_sync_waits(nc, max_waits=1, max_updates=1):
    """walrus rejects instructions with too many sync wait/update commands;
    spill excess waits onto preceding same-engine NoOps (and excess updates
    onto following ones)."""
    for f in nc.m.functions:
        for bb in f.blocks:
            il = list(bb.instructions)
            out = []
            changed = False
            for inst in il:
                si = inst.sync_info
                if si is None:
                    out.append(inst)
                    continue
                waits = list(si.on_wait)
                ups = list(si.on_update)
                pre, post = [], []
                if len(waits) > max_waits:
                    keep = waits[:max_waits]
                    for i in range(max_waits, len(waits), max_waits):
                        n = mybir.InstNoOp(
                            name=f"I-sw{nc.next_id()}", ins=[], outs=[])
                        n.engine = inst.engine
                        n.sync_info = bass_rust.SyncInfo(
                            on_wait=waits[i : i + max_waits], on_update=[])
                        pre.append(n)
                    changed = True
                else:
                    keep = waits
                if len(ups) > max_updates:
                    kup = ups[:max_updates]
                    for i in range(max_updates, len(ups), max_updates):
                        n = mybir.InstNoOp(
                            name=f"I-su{nc.next_id()}", ins=[], outs=[])
                        n.engine = inst.engine
                        n.sync_info = bass_rust.SyncInfo(
                            on_wait=[], on_update=ups[i : i + max_updates])
                        post.append(n)
                    changed = True
                else:
                    kup = ups
                if pre or post:
                    inst.sync_info = bass_rust.SyncInfo(
                        on_wait=keep, on_update=kup)
                out.extend(pre)
                out.append(inst)
                out.extend(post)
            if changed:
                bb.instructions = out

# ---------------------------------------------------------------------------
# Problem constants (hardcoded; spec: x [16, 384, 64, 64] f32, 8 heads)
# ---------------------------------------------------------------------------
NCORES = 8
BTOT, C, H, W = 16, 384, 64, 64
HEADS = 8
CP = C // HEADS  # 48
C3 = 3 * C  # 1152
NPIX = H * W  # 4096
B = BTOT // NCORES  # images per core
NCT = C // 128  # 3 channel tiles

P = 128
RS = W + 2  # padded row stride 66
PADLEN = RS * (H + 2)  # 4356
INT0 = RS + 1  # first real-pixel position 67
NST = C3 // P  # 9 channel subtiles of qkv
NTN = NPIX // 512  # 8 pixel tiles of 512
KT = NPIX // P  # 32 gram contraction tiles
NPAIR = HEADS // 2  # 4 head pairs

# conv taps: offset in padded layout for (kh, kw)
TAPS = [RS * (kh - 1) + (kw - 1) for kh in range(3) for kw in range(3)]

# PE conv row-chunks: 9 chunks of 7 rows + 1 chunk of 1 row
CHUNKS = [(7 * i, 7) for i in range(9)] + [(63, 1)]

# A^T block structure: used (in_ctile k, out_ctile m) blocks and the
# per-head pieces placed into them.
ABLOCKS = sorted(
    {
        (d // P, c // P)
        for h in range(HEADS)
        for d in (CP * h, CP * h + CP - 1)
        for c in (CP * h, CP * h + CP - 1)
    }
)


def _a_pieces():
    """Per head: list of (k, m, dlo, dhi, clo, chi) global-channel pieces."""
    out = []
    for h in range(HEADS):
        lo, hi = CP * h, CP * h + CP
        dsplit = [lo] + [P * t for t in range(1, NCT) if lo < P * t < hi] + [hi]
        for di in range(len(dsplit) - 1):
            for ci in range(len(dsplit) - 1):
                dlo, dhi = dsplit[di], dsplit[di + 1]
                clo, chi = dsplit[ci], dsplit[ci + 1]
                out.append((h, dlo // P, clo // P, dlo, dhi, clo, chi))
    return out


APIECES = _a_pieces()

# attn@v: in-ctiles contributing to each out ctile
NB = {m: sorted({k for (k, mm) in ABLOCKS if mm == m}) for m in range(NCT)}

# head -> (n2-tile pieces) for norm-scale assembly into [96, NPAIR] layout:
# head h occupies partitions 48*(h%2)..+48 of pair h//2.
def _head_pieces():
    out = {}
    for h in range(HEADS):
        lo, hi = CP * h, CP * h + CP
        pieces = []
        s0, s1 = lo // P, (hi - 1) // P
        for s in range(s0, s1 + 1):
            a = max(lo, P * s) - P * s
            b = min(hi, P * s + P) - P * s
            pieces.append((a, b, s))
        out[h] = pieces
    return out


HEAD_PIECES = _head_pieces()


def _build_nc():
    nc = bass.Bass("TRN2", target_bir_lowering=False, debug=False,
                   num_devices=NCORES)

    # ---- DRAM tensors (host pre-arranged to SBUF-shaped layouts) ----
    x_d = nc.dram_tensor("x", [B, NCT, P, NPIX], BF16, kind="ExternalInput")
    wq_d = nc.dram_tensor("wqT", [P, NCT, C3], BF16, kind="ExternalInput")
    wo_d = nc.dram_tensor("woT", [P, NCT, C], BF16, kind="ExternalInput")
    bq_d = nc.dram_tensor("bq", [P, NST], F32, kind="ExternalInput")
    bdw_d = nc.dram_tensor("bdw", [P, NST], F32, kind="ExternalInput")
    dww_d = nc.dram_tensor("dww", [P, NST, 9], F32, kind="ExternalInput")
    temp_d = nc.dram_tensor("temp", [CP, HEADS], F32, kind="ExternalInput")
    eshift_d = nc.dram_tensor("eshift", [CP, 1], F32, kind="ExternalInput")
    diag_d = nc.dram_tensor("diag", [6, 9, P, P], FP8, kind="ExternalInput")
    y_d = nc.dram_tensor("y", [B, NCT, P, NPIX], BF16, kind="ExternalOutput")
    n2q_s = nc.dram_tensor("n2q_scratch", [B, P, NCT], F32)
    n2k_s = nc.dram_tensor("n2k_scratch", [B, P, NCT], F32)
    r2_s = nc.dram_tensor("r2_scratch", [B, 1, C], F32)

    with tile.TileContext(nc) as tc:
        with (
            tc.tile_pool(name="consts", bufs=1) as consts,
            tc.tile_pool(name="xt", bufs=2) as xt_pool,
            tc.tile_pool(name="qkc", bufs=2) as qkc_pool,
            tc.tile_pool(name="tmp", bufs=2) as tmp_pool,
            tc.tile_pool(name="acc", bufs=2) as acc_pool,
            tc.tile_pool(name="perimg", bufs=1) as perimg,
            tc.tile_pool(name="smalls", bufs=2) as smalls,
            tc.tile_pool(name="ao", bufs=2) as ao_pool,
            tc.tile_pool(name="yt", bufs=2) as yt_pool,
            tc.tile_pool(name="psgemm", bufs=2, space="PSUM") as psgemm,
            tc.tile_pool(name="psconv", bufs=2, space="PSUM") as psconv,
            tc.tile_pool(name="psgram", bufs=1, space="PSUM") as psgram,
            tc.tile_pool(name="psattn", bufs=1, space="PSUM") as psattn,
            tc.tile_pool(name="psout", bufs=1, space="PSUM") as psout,
            tc.tile_pool(name="pstiny", bufs=1, space="PSUM") as pstiny,
        ):
            # ---- constants ----
            wq = consts.tile([P, NCT, C3], BF16, tag="wq")
            nc.sync.dma_start(out=wq, in_=wq_d[:])
            wo = consts.tile([P, NCT, C], BF16, tag="wo")
            nc.sync.dma_start(out=wo, in_=wo_d[:])
            bq = consts.tile([P, NST], F32, tag="bq")
            nc.sync.dma_start(out=bq, in_=bq_d[:])
            bdw = consts.tile([P, NST], F32, tag="bdw")
            nc.sync.dma_start(out=bdw, in_=bdw_d[:])
            dww = consts.tile([P, NST, 9], F32, tag="dww")
            nc.sync.dma_start(out=dww, in_=dww_d[:])
            tempt = consts.tile([CP, HEADS], F32, tag="temp")
            nc.sync.dma_start(out=tempt, in_=temp_d[:])
            eshift = consts.tile([CP, 1], F32, tag="eshift")
            nc.sync.dma_start(out=eshift, in_=eshift_d[:])
            diag8 = consts.tile([P, 6, 9, P], FP8, tag="diag8")
            nc.sync.dma_start(out=diag8, in_=diag_d.rearrange("s t p q -> p s t q"))
            ident = consts.tile([P, P], F32, tag="ident")
            make_identity(nc, ident)

            # persistent padded conv-input slots; pads zeroed once.
            qk_slots = [
                consts.tile([P, PADLEN], FP8, tag=f"qks{i}", name=f"qks{i}") for i in range(6)
            ]
            v_slots = [
                consts.tile([P, PADLEN], BF16, tag=f"vs{i}", name=f"vs{i}") for i in range(3)
            ]
            for slot in qk_slots + v_slots:
                nc.gpsimd.memset(slot[:, 0:INT0], 0.0)
                pads = bass.AP(
                    tensor=slot.tensor,
                    offset=slot.offset + (2 * RS - 1),
                    ap=[list(slot.ap[0]), [RS, H - 1], [1, 2]],
                )
                nc.gpsimd.memset(pads, 0.0)
                nc.gpsimd.memset(slot[:, RS * (H + 1) - 1 :], 0.0)

            # A^T block tiles, zeroed once (pieces overwrite same spots).
            ablk = {}
            for (k, m) in ABLOCKS:
                t = consts.tile([P, P], BF16, tag=f"ablk{k}{m}", name=f"ablk{k}{m}")
                nc.gpsimd.memset(t, 0.0)
                ablk[(k, m)] = t

            for img in range(B):
                # per-image persistent tiles
                qT = perimg.tile([P, KT, C], BF16, tag="qT")
                kT = perimg.tile([P, KT, C], BF16, tag="kT")
                vc = [
                    perimg.tile([P, NPIX], BF16, tag=f"vc{i}", name=f"vc{i}")
                    for i in range(NCT)
                ]
                n2q = perimg.tile([P, NCT], F32, tag="n2q")
                n2k = perimg.tile([P, NCT], F32, tag="n2k")

                tdest = {0: qT, 1: kT}

                # ================= 1x1 GEMM + depthwise conv =================
                for grp in range(3):  # 0: q (sts 0-2), 1: k (3-5), 2: v (6-8)
                    sts = [3 * grp + i for i in range(3)]
                    if grp < 2:
                        slots = {st: qk_slots[st - (0 if grp == 0 else 0)]
                                 for st in sts}
                        # grp0 -> qk_slots[0..2], grp1 -> qk_slots[3..5]
                        slots = {st: qk_slots[st] for st in sts}
                    else:
                        slots = {st: v_slots[st - 6] for st in sts}

                    # ---- 1x1 qkv GEMM for this group's 3 subtiles ----
                    for nt in range(NTN):
                        xt = xt_pool.tile([P, NCT, 512], BF16, tag="xt")
                        nc.sync.dma_start(
                            out=xt,
                            in_=x_d[img, :, :, 512 * nt : 512 * nt + 512]
                            .rearrange("k p n -> p k n"),
                        )
                        for st in sts:
                            ps = psgemm.tile([P, 512], F32, tag="g")
                            for k in range(NCT):
                                nc.tensor.matmul(
                                    ps,
                                    wq[:, k, P * st : P * st + P],
                                    xt[:, k, :],
                                    start=(k == 0),
                                    stop=(k == NCT - 1),
                                )
                            # strided write into padded layout (8 rows of 64)
                            dest = bass.AP(
                                tensor=slots[st].tensor,
                                offset=slots[st].offset + INT0 + 8 * RS * nt,
                                ap=[list(slots[st].ap[0]), [RS, 8], [1, W]],
                            )
                            eng = nc.scalar if (st % 2 == 0) else nc.vector
                            if eng is nc.scalar:
                                nc.scalar.activation(
                                    out=dest,
                                    in_=ps.rearrange("p (r w) -> p r w", w=W),
                                    func=AF.Identity,
                                    bias=bq[:, st : st + 1],
                                )
                            else:
                                nc.vector.tensor_scalar(
                                    out=dest,
                                    in0=ps.rearrange("p (r w) -> p r w", w=W),
                                    scalar1=bq[:, st : st + 1],
                                    scalar2=None,
                                    op0=ALU.add,
                                )

                    # ---- depthwise conv ----
                    for st in sts:
                        slot = slots[st]
                        if grp < 2:
                            # --- PE fp8 DoubleRow conv, 10 row-chunks ---
                            cdst = qkc_pool.tile([P, NPIX], BF16, tag="qkc")
                            for ci, (r0, nr) in enumerate(CHUNKS):
                                pos0 = RS * r0 + INT0
                                # last chunk: stay inside the padded buffer
                                # for the +67 tap (trailing pads unused)
                                ncol = min(RS * nr, PADLEN - pos0 - TAPS[8])
                                ps = psconv.tile([P, 462], F32, tag="c")
                                for pair in range(4):
                                    ta, tb = TAPS[2 * pair], TAPS[2 * pair + 1]
                                    rhs = bass.AP(
                                        tensor=slot.tensor,
                                        offset=slot.offset + pos0 + ta,
                                        ap=[list(slot.ap[0]), [tb - ta, 2],
                                            [1, ncol]],
                                    )
                                    lhsT = bass.AP(
                                        tensor=diag8.tensor,
                                        offset=diag8.offset
                                        + (st * 9 + 2 * pair) * P,
                                        ap=[list(diag8.ap[0]), [P, 2], [1, P]],
                                    )
                                    nc.tensor.matmul(
                                        ps[:, :ncol], lhsT, rhs,
                                        start=(pair == 0), stop=False,
                                        perf_mode=PM.DoubleRow,
                                    )
                                nc.tensor.matmul(
                                    ps[:, :ncol],
                                    diag8[:, st, 8, :],
                                    slot[:, pos0 + TAPS[8] :
                                         pos0 + TAPS[8] + ncol],
                                    start=False, stop=True,
                                )
                                # drain strided psum -> compact bf16 + bias
                                pssrc = bass.AP(
                                    tensor=ps.tensor, offset=ps.offset,
                                    ap=[list(ps.ap[0]), [RS, nr], [1, W]],
                                )
                                dst = cdst[:, W * r0 : W * (r0 + nr)]
                                if ci % 2 == 0:
                                    nc.scalar.activation(
                                        out=dst.rearrange(
                                            "p (r w) -> p r w", w=W),
                                        in_=pssrc, func=AF.Identity,
                                        bias=bdw[:, st : st + 1],
                                    )
                                else:
                                    nc.vector.tensor_scalar(
                                        out=dst.rearrange(
                                            "p (r w) -> p r w", w=W),
                                        in0=pssrc,
                                        scalar1=bdw[:, st : st + 1],
                                        scalar2=None, op0=ALU.add,
                                    )

                            # transpose first (reads original values), then
                            # square in place for the per-channel norm
                            s = st - 3 * grp
                            n2 = n2q if grp == 0 else n2k
                            eng = nc.sync if s % 2 == 0 else nc.scalar
                            eng.dma_start_transpose(
                                tdest[grp][:, :, P * s : P * s + P], cdst[:]
                            )
                            nc.scalar.activation(
                                out=cdst[:], in_=cdst[:], func=AF.Square,
                                accum_out=n2[:, s : s + 1],
                            )
                        else:
                            # --- v conv: DVE adds + Scalar/GpSimd mults ---
                            ct = st - 6
                            for hf in range(2):
                                r0 = 32 * hf
                                base = slot.offset + RS * r0 + INT0

                                def view3(toff, t=slot):
                                    return bass.AP(
                                        tensor=t.tensor,
                                        offset=base + toff,
                                        ap=[list(t.ap[0]), [RS, 32], [1, W]],
                                    )

                                acc = acc_pool.tile([P, 2048], BF16,
                                                    tag="acc")
                                acc3 = acc.rearrange("p (r w) -> p r w", w=W)
                                # tap 0 on DVE (mult + bias)
                                nc.vector.tensor_scalar(
                                    out=acc3, in0=view3(TAPS[0]),
                                    scalar1=dww[:, st, 0:1],
                                    scalar2=bdw[:, st : st + 1],
                                    op0=ALU.mult, op1=ALU.add,
                                )
                                for t in range(1, 9):
                                    tmp = tmp_pool.tile([P, 2048], BF16,
                                                        tag="tmp")
                                    tmp3 = tmp.rearrange(
                                        "p (r w) -> p r w", w=W)
                                    if t % 2 == 1:
                                        nc.scalar.activation(
                                            out=tmp3, in_=view3(TAPS[t]),
                                            func=AF.Copy,
                                            scale=dww[:, st, t : t + 1],
                                        )
                                    else:
                                        nc.gpsimd.tensor_scalar(
                                            out=tmp3, in0=view3(TAPS[t]),
                                            scalar1=dww[:, st, t : t + 1],
                                            scalar2=None, op0=ALU.mult,
                                        )
                                    if t < 8:
                                        nc.vector.tensor_tensor(
                                            out=acc3, in0=acc3, in1=tmp3,
                                            op=ALU.add,
                                        )
                                    else:
                                        dst = vc[ct][:, 2048 * hf :
                                                     2048 * hf + 2048]
                                        nc.vector.tensor_tensor(
                                            out=dst.rearrange(
                                                "p (r w) -> p r w", w=W),
                                            in0=acc3, in1=tmp3, op=ALU.add,
                                        )

                # ================= norm scales =================
                qh2 = smalls.tile([CP, HEADS], F32, tag="qh2")
                r2 = smalls.tile([1, C], F32, tag="r2")
                nc.sync.dma_start(out=n2q_s[img], in_=n2q[:])
                nc.sync.dma_start(out=n2k_s[img], in_=n2k[:])
                for h in range(HEADS):
                    off = 0
                    for (a, b, s) in HEAD_PIECES[h]:
                        ln = b - a
                        nc.sync.dma_start(
                            out=qh2[off : off + ln, h : h + 1],
                            in_=n2q_s[img, a:b, s : s + 1],
                        )
                        nc.scalar.dma_start(
                            out=r2[0:1, CP * h + off : CP * h + off + ln],
                            in_=n2k_s[img, a:b, s : s + 1]
                            .rearrange("p o -> o p"),
                        )
                        off += ln
                # rqh = temp / sqrt(qh2); ck = 1/sqrt(r2)
                rqh = smalls.tile([CP, HEADS], F32, tag="rqh")
                nc.scalar.activation(out=qh2, in_=qh2, func=AF.Sqrt)
                nc.vector.reciprocal(out=qh2, in_=qh2)
                nc.vector.tensor_tensor(out=rqh, in0=qh2, in1=tempt,
                                        op=ALU.mult)
                nc.scalar.activation(out=r2, in_=r2, func=AF.Sqrt)
                nc.vector.reciprocal(out=r2, in_=r2)
                ck = smalls.tile([CP, C], F32, tag="ck")
                nc.sync.dma_start(out=r2_s[img], in_=r2[:])
                nc.scalar.dma_start(
                    out=ck,
                    in_=bass.AP(tensor=r2_s, offset=img * C,
                                ap=[[0, CP], [1, C]]),
                )

                # ================= Gram + softmax =================
                S = smalls.tile([CP, HEADS, CP], F32, tag="S")
                for h in range(HEADS):
                    ps = psgram.tile([CP, CP], F32, tag="gram")
                    for kt in range(KT):
                        nc.tensor.matmul(
                            ps,
                            qT[:, kt, CP * h : CP * h + CP],
                            kT[:, kt, CP * h : CP * h + CP],
                            start=(kt == 0),
                            stop=(kt == KT - 1),
                        )
                    nc.vector.tensor_scalar_mul(
                        S[:, h, :], ps, rqh[:, h : h + 1],
                    )

                # logits * ck, exp with constant shift, normalize
                ckv = ck.rearrange("p (h d) -> p h d", h=HEADS)
                nc.vector.tensor_tensor(out=S, in0=S, in1=ckv, op=ALU.mult)
                nc.scalar.activation(out=S, in_=S, func=AF.Exp, bias=eshift)
                sm = smalls.tile([CP, HEADS], F32, tag="sm")
                nc.vector.tensor_reduce(out=sm, in_=S, axis=AX.X, op=ALU.add)
                nc.vector.reciprocal(out=sm, in_=sm)
                nc.vector.tensor_tensor(
                    out=S, in0=S, in1=sm[:, :, None].to_broadcast(S.shape),
                    op=ALU.mult,
                )

                # ================= A^T assembly =================
                ut = smalls.tile([CP, HEADS, CP], BF16, tag="ut")
                for h in range(HEADS):
                    pst = pstiny.tile([CP, CP], F32, tag="tr")
                    nc.tensor.transpose(
                        pst, S[:, h, :], ident[0:CP, 0:CP]
                    )
                    nc.vector.tensor_copy(out=ut[:, h, :], in_=pst)
                for (h, k, m, dlo, dhi, clo, chi) in APIECES:
                    nc.sync.dma_start(
                        out=ablk[(k, m)][dlo - P * k : dhi - P * k,
                                         clo - P * m : chi - P * m],
                        in_=ut[dlo - CP * h : dhi - CP * h, h,
                               clo - CP * h : chi - CP * h],
                    )

                # ================= attn @ v -> out-projection =================
                for nt in range(NTN):
                    aot = ao_pool.tile([P, NCT, 512], BF16, tag="ao")
                    for m in range(NCT):
                        ps = psattn.tile([P, 512], F32, tag="av")
                        ks = NB[m]
                        for i, k in enumerate(ks):
                            nc.tensor.matmul(
                                ps,
                                ablk[(k, m)][:],
                                vc[k][:, 512 * nt : 512 * nt + 512],
                                start=(i == 0),
                                stop=(i == len(ks) - 1),
                            )
                        if m % 2 == 0:
                            nc.scalar.activation(out=aot[:, m, :], in_=ps,
                                                 func=AF.Identity)
                        else:
                            nc.vector.tensor_copy(out=aot[:, m, :], in_=ps)
                    for mo in range(NCT):
                        ps = psout.tile([P, 512], F32, tag="o")
                        for k in range(NCT):
                            nc.tensor.matmul(
                                ps,
                                wo[:, k, P * mo : P * mo + P],
                                aot[:, k, :],
                                start=(k == 0),
                                stop=(k == NCT - 1),
                            )
                        yt = yt_pool.tile([P, 512], BF16, tag="yt")
                        if mo % 2 == 0:
                            nc.vector.tensor_copy(out=yt, in_=ps)
                        else:
                            nc.scalar.activation(out=yt, in_=ps,
                                                 func=AF.Identity)
                        nc.sync.dma_start(
                            out=y_d[img, mo, :, 512 * nt : 512 * nt + 512],
                            in_=yt,
                        )

    _split_sync_waits(nc)
    return nc


_CACHE = {}


def kernel(x, W_qkv, b_qkv, W_dw, b_dw, W_out, b_out, temperature):
    x = np.asarray(x, np.float32)
    W_qkv = np.asarray(W_qkv, np.float32)
    b_qkv = np.asarray(b_qkv, np.float32)
    W_dw = np.asarray(W_dw, np.float32)
    b_dw = np.asarray(b_dw, np.float32)
    W_out = np.asarray(W_out, np.float32)
    b_out = np.asarray(b_out, np.float32)
    temperature = np.asarray(temperature, np.float32)

    if "nc" not in _CACHE:
        _CACHE["nc"] = _build_nc()
    nc = _CACHE["nc"]

    # ---- host-side prep into SBUF-shaped layouts ----
    wqT = np.ascontiguousarray(
        W_qkv.T.reshape(NCT, P, C3).transpose(1, 0, 2)
    ).astype(ml_dtypes.bfloat16)  # [128, 3, 1152]
    woT = np.ascontiguousarray(
        W_out.T.reshape(NCT, P, C).transpose(1, 0, 2)
    ).astype(ml_dtypes.bfloat16)  # [128, 3, 384]
    bq = np.ascontiguousarray(b_qkv.reshape(NST, P).T)  # [128, 9]
    bdw = np.ascontiguousarray(b_dw.reshape(NST, P).T)  # [128, 9]
    taps = W_dw.reshape(C3, 9)  # [1152, 9] in (kh, kw) order
    dww = np.ascontiguousarray(
        taps.reshape(NST, P, 9).transpose(1, 0, 2)
    )  # [128, 9, 9]
    tb = temperature.reshape(HEADS)
    temp = np.broadcast_to(tb[None, :], (CP, HEADS)).astype(np.float32).copy()
    eshift = np.full((CP, 1), -float(np.abs(tb).max()), np.float32)
    # fp8 diagonal tap matrices for q,k subtiles 0..5
    diag = np.zeros((6, 9, P, P), np.float32)
    ar = np.arange(P)
    for st in range(6):
        for t in range(9):
            diag[st, t, ar, ar] = taps[P * st : P * st + P, t]
    diag8 = diag.astype(ml_dtypes.float8_e4m3)

    xr = x.reshape(BTOT, NCT, P, NPIX).astype(ml_dtypes.bfloat16)

    base = {
        "wqT": wqT, "woT": woT, "bq": bq, "bdw": bdw,
        "dww": dww, "temp": temp, "eshift": eshift, "diag": diag8,
    }
    in_maps = []
    for core in range(NCORES):
        m = dict(base)
        m["x"] = np.ascontiguousarray(xr[B * core : B * core + B])
        in_maps.append(m)

    res = run_bass_kernel_spmd(nc, in_maps, list(range(NCORES)),
                               trace=bool(os.environ.get("KERNEL_TRACE")))
    if os.environ.get("KERNEL_TRACE"):
        _CACHE["exec_time_ns"] = res.exec_time_ns

    outs = [
        res.results[c]["y"].astype(np.float32).reshape(B, C, H, W)
        for c in range(NCORES)
    ]
    y = np.concatenate(outs, axis=0)
    y += b_out[None, :, None, None]
    return y
